# revision 1
# baseline (speedup 1.0000x reference)
"""BertMoELayer (B=4, S=2048, H=768, F=3072, E=8, top-2) on 8 Trainium2 cores.

Sharding strategy (per the problem's sharding hint): expert-parallel with
token dispatch by top-k expert index. Expert weights are sharded one expert
per core; the host evaluates the router only to DECIDE the shard assignment
(which tokens go to which core) and gathers each core's token subset, padded
to a static capacity. All numeric computation of the layer runs on device:

  per core c, over its gathered tokens (capacity C):
    logits = x.T @ Wr.T            (fp32 matmul, on device)
    w_c    = top-2 softmax weight of expert c     (on device, fp32)
    hT     = gelu(Wi[c].T^T @ x.T + bi[c])        (bf16 matmul, fp32 psum)
    out_c  = w_c * (hT^T @ Wo[c].T + bo[c])       (bf16 matmul, fp32 psum)

The host unshards by scatter-adding each core's (already weighted) rows:
out[token_list_c] += out_c. Padding rows are dropped (never scattered).

Matmul FLOPs run in bf16 with fp32 accumulation; the router runs entirely in
fp32 so the top-2 selection/weights match the fp32 reference. All tensors are
host-pre-transposed so every DMA is a natural row-major strided load.
"""

import numpy as np
import ml_dtypes

import concourse.bass as bass
import concourse.tile as tile
from concourse import bacc, mybir
from concourse.bass_utils import run_bass_kernel_spmd
from concourse.masks import make_identity

B, S, H, F, E = 4, 2048, 768, 3072, 8
T = B * S
N_CORES = 8
TOP_K = 2

P = 128          # SBUF partitions
TB = 512         # token block (matmul free dim)
KH = H // P      # 6   h-chunks
KF = F // P      # 24  f-chunks
HO = 384         # output free-dim split (2 x 384 = 768)

F32 = mybir.dt.float32
BF16 = mybir.dt.bfloat16
BF16_NP = ml_dtypes.bfloat16


def build_nc(cap: int, tb: int = TB):
    """Per-core program: router weight + dense expert FFN over `cap` tokens."""
    assert cap % P == 0 and tb % P == 0
    # variable-size token blocks: full tb-sized blocks plus a 128-multiple tail
    blocks = []
    t0 = 0
    while t0 < cap:
        b = min(tb, cap - t0)
        blocks.append((t0, b))
        t0 += b

    # Bacc (not plain Bass): its compile() pass splits multi-wait instructions
    # into event-semaphore chains, which walrus requires (max 1 wait per inst).
    nc = bacc.Bacc(None)

    xgf = nc.declare_dram_parameter("xgf", [H, cap], F32, isOutput=False)
    xg = nc.declare_dram_parameter("xg", [H, cap], BF16, isOutput=False)
    wiT = nc.declare_dram_parameter("wiT", [H, F], BF16, isOutput=False)
    woT = nc.declare_dram_parameter("woT", [F, H], BF16, isOutput=False)
    wrT = nc.declare_dram_parameter("wrT", [H, E], F32, isOutput=False)
    bi = nc.declare_dram_parameter("bi", [F], F32, isOutput=False)
    bo = nc.declare_dram_parameter("bo", [H], F32, isOutput=False)
    esel = nc.declare_dram_parameter("esel", [E], F32, isOutput=False)
    out = nc.declare_dram_parameter("out", [cap, H], F32, isOutput=True)

    # Row-major DRAM views with the contraction dim chunked onto partitions.
    xgf_r = xgf.rearrange("(k p) t -> p k t", p=P)    # [128, KH, cap]
    xg_r = xg.rearrange("(k p) t -> p k t", p=P)      # [128, KH, cap]
    wiT_r = wiT.rearrange("(k p) f -> p k f", p=P)    # [128, KH, F]
    woT_r = woT.rearrange("(j p) f -> p j f", p=P)    # [128, KF, H]
    wrT_r = wrT.rearrange("(k p) e -> p k e", p=P)    # [128, KH, E]
    bi_r = bi.rearrange("(j p) -> p j", p=P)          # [128, KF]
    out_r = out.rearrange("(n p) h -> n p h", p=P)    # [cap/128, 128, H]

    with tile.TileContext(nc) as tc:
        with (
            tc.tile_pool(name="weights", bufs=1) as wpool,
            tc.tile_pool(name="xin", bufs=2) as xpool,
            tc.tile_pool(name="hbuf", bufs=2) as hpool,
            tc.tile_pool(name="obuf", bufs=3) as opool,
            tc.tile_pool(name="router", bufs=2) as rpool,
            tc.tile_pool(name="psum_h", bufs=3, space="PSUM") as ph_pool,
            tc.tile_pool(name="psum_o", bufs=3, space="PSUM") as po_pool,
            tc.tile_pool(name="psum_r", bufs=1, space="PSUM") as pr_pool,
            tc.tile_pool(name="psum_rt", bufs=1, space="PSUM") as prt_pool,
        ):
            # ---- preamble, ordered so block-0 compute can start early ----
            # block-0 tokens + router weights + first Wi column-group first;
            # the rest of the weights stream in under compute. Wi is split into
            # independent group tiles so each mm1 chain only depends on the one
            # DMA that carries its columns.
            WG = 4  # j-columns per Wi group tile
            b0 = blocks[0][1]
            # the first mm1 chain depends only on x0_bf + wig0: issue those two
            # first so they land on separate queues and stream in parallel
            x0_bf = xpool.tile([P, KH, b0], BF16, tag="xb")
            nc.sync.dma_start(out=x0_bf, in_=xg_r[:, :, 0:b0])
            wi_groups = [
                wpool.tile([P, KH, WG * P], BF16, tag=f"wig{g}", name=f"wig{g}")
                for g in range(KF // WG)
            ]
            nc.sync.dma_start(out=wi_groups[0], in_=wiT_r[:, :, 0 : WG * P])
            x0_f32 = xpool.tile([P, KH, b0], F32, tag="xf")
            nc.sync.dma_start(out=x0_f32, in_=xgf_r[:, :, 0:b0])
            wrT_sb = wpool.tile([P, KH, E], F32)
            nc.sync.dma_start(out=wrT_sb, in_=wrT_r)
            bi_sb = wpool.tile([P, KF], F32)
            nc.sync.dma_start(out=bi_sb, in_=bi_r)
            for g in range(1, KF // WG):
                nc.sync.dma_start(
                    out=wi_groups[g], in_=wiT_r[:, :, g * WG * P : (g + 1) * WG * P]
                )

            woT_sb = wpool.tile([P, KF, H], BF16)
            for g in range(0, KF, 4):
                nc.sync.dma_start(
                    out=woT_sb[:, g : g + 4, :], in_=woT_r[:, g : g + 4, :]
                )
            # bo broadcast to all 128 partitions (it is added along the free dim)
            bo_sb = wpool.tile([P, H], F32)
            nc.gpsimd.dma_start(out=bo_sb, in_=bo[None, :].to_broadcast([P, H]))
            # one-hot expert selector, broadcast to all partitions
            esel_sb = wpool.tile([P, E], F32)
            nc.gpsimd.dma_start(out=esel_sb, in_=esel[None, :].to_broadcast([P, E]))
            # identity for the PE-mode transpose of the router logits
            id8 = wpool.tile([E, E], F32, name="id8")
            make_identity(nc, id8)

            def router_logits(x_f32, b):
                # one chain per block: logits^T [E, b] (lhsT=wrT -> 8-column
                # weight loads, full-width rhs streaming)
                pslT = pr_pool.tile([E, b], F32, tag="pr")
                for k in range(KH):
                    nc.tensor.matmul(
                        pslT,
                        lhsT=wrT_sb[:, k, :],
                        rhs=x_f32[:, k, :],
                        start=(k == 0),
                        stop=(k == KH - 1),
                    )
                lgT_sb = rpool.tile([E, b], F32, tag="lgT")
                nc.vector.tensor_copy(out=lgT_sb, in_=pslT)
                return lgT_sb

            def router_chain(ts, lgT_sb, w_blk):
                # transpose this ts's logits back to [t, e], then
                # top-2 softmax -> this expert's weight (all fp32)
                pst = prt_pool.tile([P, E], F32, tag="prt")
                nc.tensor.transpose(pst, lgT_sb[:, ts * P : (ts + 1) * P], id8)
                lg = rpool.tile([P, E], F32, tag="lg")
                nc.vector.tensor_copy(out=lg, in_=pst)
                m1 = rpool.tile([P, 1], F32, tag="m1")
                nc.vector.reduce_max(m1, lg, axis=mybir.AxisListType.X)
                # mask out the argmax, then find the 2nd max
                ge = rpool.tile([P, E], F32, tag="ge")
                nc.vector.tensor_scalar(
                    ge, lg, scalar1=m1, scalar2=-1e30,
                    op0=mybir.AluOpType.is_ge, op1=mybir.AluOpType.mult,
                )
                mk = rpool.tile([P, E], F32, tag="mk")
                nc.vector.tensor_tensor(mk, lg, ge, op=mybir.AluOpType.add)
                m2 = rpool.tile([P, 1], F32, tag="m2")
                nc.vector.reduce_max(m2, mk, axis=mybir.AxisListType.X)
                # softmax over {m1, m2}: w1 = 1/(1+e2), w2 = e2/(1+e2)
                dd = rpool.tile([P, 1], F32, tag="dd")
                nc.vector.tensor_tensor(dd, m2, m1, op=mybir.AluOpType.subtract)
                e2 = rpool.tile([P, 1], F32, tag="e2")
                nc.scalar.activation(e2, dd, mybir.ActivationFunctionType.Exp)
                den = rpool.tile([P, 1], F32, tag="den")
                nc.vector.tensor_scalar_add(den, e2, 1.0)
                w1 = rpool.tile([P, 1], F32, tag="w1")
                nc.vector.reciprocal(w1, den)
                w2 = rpool.tile([P, 1], F32, tag="w2")
                nc.vector.tensor_tensor(w2, e2, w1, op=mybir.AluOpType.mult)
                # this core's logit: lc = sum(lg * esel)
                lc_t = rpool.tile([P, E], F32, tag="lct")
                nc.vector.tensor_tensor(lc_t, lg, esel_sb, op=mybir.AluOpType.mult)
                lc = rpool.tile([P, 1], F32, tag="lc")
                nc.vector.reduce_sum(lc, lc_t, axis=mybir.AxisListType.X)
                d1 = rpool.tile([P, 1], F32, tag="d1")
                nc.vector.tensor_scalar(
                    d1, lc, scalar1=m1, scalar2=None, op0=mybir.AluOpType.is_ge
                )
                g2 = rpool.tile([P, 1], F32, tag="g2")
                nc.vector.tensor_scalar(
                    g2, lc, scalar1=m2, scalar2=None, op0=mybir.AluOpType.is_ge
                )
                # w = d1*(w1-w2) + g2*w2
                wa = rpool.tile([P, 1], F32, tag="wa")
                nc.vector.tensor_tensor(wa, w1, w2, op=mybir.AluOpType.subtract)
                t1 = rpool.tile([P, 1], F32, tag="t1")
                nc.vector.tensor_tensor(t1, d1, wa, op=mybir.AluOpType.mult)
                t2 = rpool.tile([P, 1], F32, tag="t2")
                nc.vector.tensor_tensor(t2, g2, w2, op=mybir.AluOpType.mult)
                nc.vector.tensor_tensor(
                    w_blk[:, ts : ts + 1], t1, t2, op=mybir.AluOpType.add
                )

            for t0, b in blocks:
                nts = b // P
                if t0 == 0:
                    x_f32, x_bf = x0_f32, x0_bf
                else:
                    x_f32 = xpool.tile([P, KH, b], F32, tag="xf")
                    nc.sync.dma_start(out=x_f32, in_=xgf_r[:, :, t0 : t0 + b])
                    x_bf = xpool.tile([P, KH, b], BF16, tag="xb")
                    nc.sync.dma_start(out=x_bf, in_=xg_r[:, :, t0 : t0 + b])

                w_blk = rpool.tile([P, nts], F32, tag="w")

                # ---- layer 1: hT[f, t] = gelu(WiT^T @ xT + bi), with the
                # router chains interleaved between the dense j-chains so the
                # PE activity stays dense (no HAM re-throttle windows) ----
                hT = hpool.tile([P, KF, b], BF16, tag="hT")
                for j in range(KF):
                    ps = ph_pool.tile([P, b], F32, tag="ph")
                    wig = wi_groups[j // WG]
                    jj = j % WG
                    for k in range(KH):
                        nc.tensor.matmul(
                            ps,
                            lhsT=wig[:, k, jj * P : (jj + 1) * P],
                            rhs=x_bf[:, k, :],
                            start=(k == 0),
                            stop=(k == KH - 1),
                        )
                    nc.scalar.activation(
                        out=hT[:, j, :],
                        in_=ps,
                        func=mybir.ActivationFunctionType.Gelu,
                        bias=bi_sb[:, j : j + 1],
                        scale=1.0,
                    )
                    if j == 0:
                        lgT_sb = router_logits(x_f32, b)
                    elif j - 1 < nts:
                        router_chain(j - 1, lgT_sb, w_blk)

                # ---- layer 2 + bo + routing-weight scale ----
                for ts in range(nts):
                    po_a = po_pool.tile([P, HO], F32, tag="po")
                    po_b = po_pool.tile([P, HO], F32, tag="po")
                    for j in range(KF):
                        lhsT = hT[:, j, ts * P : (ts + 1) * P]
                        nc.tensor.matmul(
                            po_a, lhsT=lhsT, rhs=woT_sb[:, j, 0:HO],
                            start=(j == 0), stop=(j == KF - 1),
                        )
                        nc.tensor.matmul(
                            po_b, lhsT=lhsT, rhs=woT_sb[:, j, HO : 2 * HO],
                            start=(j == 0), stop=(j == KF - 1),
                        )
                    o_sb = opool.tile([P, H], F32, tag="os")
                    wcol = w_blk[:, ts : ts + 1]
                    nc.vector.tensor_tensor(
                        o_sb[:, 0:HO], po_a, bo_sb[:, 0:HO], op=mybir.AluOpType.add
                    )
                    nc.vector.tensor_tensor(
                        o_sb[:, HO : 2 * HO], po_b, bo_sb[:, HO : 2 * HO],
                        op=mybir.AluOpType.add,
                    )
                    nc.vector.tensor_scalar_mul(o_sb, o_sb, scalar1=wcol)
                    nc.sync.dma_start(out=out_r[t0 // P + ts], in_=o_sb)

    nc.compile()
    return nc


_NC_CACHE: dict = {}


def _get_nc(cap: int):
    if cap not in _NC_CACHE:
        _NC_CACHE[cap] = build_nc(cap)
    return _NC_CACHE[cap]


def _ensure_axon_hooks_module():
    """run_bass_kernel_spmd(trace=True) (e.g. via env BASS_TRACE=1) imports
    antenv.axon_hooks, which some images lack even though the boot code that
    would register the NTFF hook is present. Provide the module and register
    the real hook when available so tracing works instead of crashing."""
    try:
        import antenv.axon_hooks  # noqa: F401

        return
    except ImportError:
        pass
    try:
        import sys
        import types

        import antenv  # noqa: F401

        mod = types.ModuleType("antenv.axon_hooks")
        state = {"hook": None}
        mod.set_axon_ntff_profile_hook = lambda h: state.__setitem__("hook", h)
        mod.get_axon_ntff_profile_hook = lambda: state["hook"]
        try:
            from trn_agent_boot.trn_boot import _ntff_profile_via_ctypes

            mod.set_axon_ntff_profile_hook(
                _ntff_profile_via_ctypes("/opt/axon/libaxon_pjrt.so")
            )
        except Exception:
            pass
        sys.modules["antenv.axon_hooks"] = mod
    except Exception:
        pass


def _shard_tokens(xf, Wr):
    """Host-side sharding function: top-2 expert index per token (matches
    jax.lax.top_k tie-breaking: lowest index wins on ties)."""
    logits = xf.astype(np.float32) @ np.asarray(Wr, np.float32).T  # [T, E]
    i1 = np.argmax(logits, axis=1)
    l2 = logits.copy()
    l2[np.arange(len(i1)), i1] = -np.inf
    i2 = np.argmax(l2, axis=1)
    tokens = np.arange(logits.shape[0])
    tok_lists = []
    for c in range(N_CORES):
        tok_lists.append(np.concatenate([tokens[i1 == c], tokens[i2 == c]]))
    return tok_lists


def kernel(x, Wr, Wi, bi, Wo, bo, _trace=False):
    x = np.asarray(x)
    xf = x.reshape(-1, H).astype(np.float32)
    tok_lists = _shard_tokens(xf, Wr)
    maxc = max(len(tl) for tl in tok_lists)
    cap = max(P, int(np.ceil(maxc / P) * P))

    xT = np.ascontiguousarray(xf.T)  # [H, T] fp32
    in_maps = []
    for c in range(N_CORES):
        tl = tok_lists[c]
        xgf = np.zeros((H, cap), dtype=np.float32)
        xgf[:, : len(tl)] = xT[:, tl]
        sel = np.zeros(E, np.float32)
        sel[c] = 1.0
        in_maps.append(
            {
                "xgf": xgf,
                "xg": xgf.astype(BF16_NP),
                "wiT": np.ascontiguousarray(np.asarray(Wi[c], np.float32).T).astype(
                    BF16_NP
                ),
                "woT": np.ascontiguousarray(np.asarray(Wo[c], np.float32).T).astype(
                    BF16_NP
                ),
                "wrT": np.ascontiguousarray(np.asarray(Wr, np.float32).T),
                "bi": np.asarray(bi[c], np.float32),
                "bo": np.asarray(bo[c], np.float32),
                "esel": sel,
            }
        )

    _ensure_axon_hooks_module()
    nc = _get_nc(cap)
    res = run_bass_kernel_spmd(
        nc, in_maps, core_ids=list(range(N_CORES)), trace=_trace
    )

    # Unshard: scatter-add the per-expert (already routing-weighted) rows.
    out = np.zeros((T, H), dtype=np.float32)
    for c in range(N_CORES):
        tl = tok_lists[c]
        out[tl] += res.results[c]["out"][: len(tl)]
    out = out.reshape(x.shape)
    if _trace:
        return out, res
    return out



# revision 6
# speedup vs baseline: 1.0206x; 1.0206x over previous
"""BertMoELayer (B=4, S=2048, H=768, F=3072, E=8, top-2) on 8 Trainium2 cores.

Expert-parallel: one expert per core; the host evaluates the router in fp32
only to DECIDE the shard assignment (which tokens go to which core, matching
jax.lax.top_k tie-breaking) and gathers each core's token subset. All numeric
computation of the layer runs on device:

  per core c, over its gathered tokens (capacity = max expert load, exact):
    logitsT = WrT^T @ xT                     (bf16 matmul, fp32 psum)
    w_c     = 1 / (exp(m1-lc) + exp(m2-lc))  (smooth top-2 softmax weight;
                                              m1/m2 = top-2 of this token's
                                              logits, lc = this expert's logit)
    hT      = gelu(WiT^T @ xT + bi[c])       (bf16 matmul, fp32 psum)
    out_c   = w_c * (hT^T @ WoT + bo[c])     (bf16 matmul, fp32 psum)

The smooth w formula has no comparison cliffs: when bf16 logits reorder a
near-tie relative to the host's fp32 selection, the weight degrades
continuously (the swapped logits are equal to within the noise), so no
selection-consistency hazard exists between host and device. Logits are
bit-identical across cores (same k-chain accumulation order), so the two
selected cores' weights sum to exactly softmax's 1.

The host unshards by scatter-adding each core's (already weighted) rows.
"""

import numpy as np
import ml_dtypes

import concourse.bass as bass
import concourse.tile as tile
from concourse import bacc, mybir
from concourse.bass_utils import run_bass_kernel_spmd
from concourse.masks import make_identity

B, S, H, F, E = 4, 2048, 768, 3072, 8
T = B * S
N_CORES = 8
TOP_K = 2

P = 128          # SBUF partitions
KH = H // P      # 6   h-chunks
KF = F // P      # 24  f-chunks
HO = 384         # output free-dim split (2 x 384 = 768)

F32 = mybir.dt.float32
BF16 = mybir.dt.bfloat16
BF16_NP = ml_dtypes.bfloat16

# wi column groups (in j units of 128): small first group so the first
# L1 chain's weight DMA lands quickly at startup
WI_GROUPS = (1, 4, 4, 4, 4, 4, 3)


def make_blocks(cap: int):
    """Token blocks: small first block for fast start, 512-blocks in the
    middle, tail <= 512. All blocks except the last are multiples of 128
    (the batched out-DMA view indexes 128-row DRAM tiles by block start);
    non-tail blocks are >= 256 so L1 chains stay matmul-bound."""
    assert cap >= 512
    blocks = [256]
    rem = cap - 256
    while rem > 512:
        blocks.append(512)
        rem -= 512
    if rem <= 128 and blocks[-1] == 512:
        # avoid a tiny LDWEIGHTS-bound tail: rebalance the last 512
        blocks[-1] = 256
        rem += 256
    blocks.append(rem)
    assert sum(blocks) == cap
    assert all(b % 128 == 0 for b in blocks[:-1]) and blocks[-1] <= 512
    return blocks


def build_nc(cap: int):
    """Per-core program: router weight + dense expert FFN over `cap` tokens."""
    blocks = make_blocks(cap)
    cap_pad = ((cap + P - 1) // P) * P  # out is padded to x128 for clean DMA views

    # Bacc (not plain Bass): its compile() pass splits multi-wait instructions
    # into event-semaphore chains, which walrus requires (max 1 wait per inst).
    nc = bacc.Bacc(None)

    xg = nc.declare_dram_parameter("xg", [H, cap], BF16, isOutput=False)
    wiT = nc.declare_dram_parameter("wiT", [H, F], BF16, isOutput=False)
    woT = nc.declare_dram_parameter("woT", [F, H], BF16, isOutput=False)
    wrT = nc.declare_dram_parameter("wrT", [H, E], BF16, isOutput=False)
    bi = nc.declare_dram_parameter("bi", [F], F32, isOutput=False)
    bo = nc.declare_dram_parameter("bo", [H], F32, isOutput=False)
    esel = nc.declare_dram_parameter("esel", [E], F32, isOutput=False)
    out = nc.declare_dram_parameter("out", [cap_pad, H], F32, isOutput=True)

    # Row-major DRAM views with the contraction dim chunked onto partitions.
    xg_r = xg.rearrange("(k p) t -> p k t", p=P)      # [128, KH, cap]
    wiT_r = wiT.rearrange("(k p) f -> p k f", p=P)    # [128, KH, F]
    woT_r = woT.rearrange("(j p) f -> p j f", p=P)    # [128, KF, H]
    wrT_r = wrT.rearrange("(k p) e -> p k e", p=P)    # [128, KH, E]
    bi_r = bi.rearrange("(j p) -> p j", p=P)          # [128, KF]
    out_r = out.rearrange("(n p) h -> p n h", p=P)    # [128, cap_pad/128, H]

    # j (0..23) -> (wi group tile index, local column slice)
    j_map = []
    g0 = 0
    for gi, gw in enumerate(WI_GROUPS):
        for jj in range(gw):
            j_map.append((gi, jj))
        g0 += gw

    with tile.TileContext(nc) as tc:
        with (
            tc.tile_pool(name="weights", bufs=1) as wpool,
            tc.tile_pool(name="xin", bufs=3) as xpool,
            tc.tile_pool(name="hbuf", bufs=2) as hpool,
            tc.tile_pool(name="obuf", bufs=2) as opool,
            tc.tile_pool(name="router", bufs=2) as rpool,
            tc.tile_pool(name="psum_h", bufs=3, space="PSUM") as ph_pool,
            tc.tile_pool(name="psum_o", bufs=3, space="PSUM") as po_pool,
            tc.tile_pool(name="psum_r", bufs=1, space="PSUM") as pr_pool,
            tc.tile_pool(name="psum_rt", bufs=1, space="PSUM") as prt_pool,
        ):
            # ---- preamble: interleave weight/x DMAs across the two HWDGE
            # rings (sync + scalar) so block-0 compute can start early and the
            # wi group stream stays ahead of the L1 chains ----
            b0 = blocks[0]
            x0_bf = xpool.tile([P, KH, b0], BF16, tag="xb", name="x0_bf")
            x_tiles = {0: x0_bf}
            # first x block: two half-loads on separate rings
            nc.sync.dma_start(out=x_tiles[0][:, :, 0 : b0 // 2], in_=xg_r[:, :, 0 : b0 // 2])
            nc.scalar.dma_start(out=x_tiles[0][:, :, b0 // 2 : b0], in_=xg_r[:, :, b0 // 2 : b0])

            wi_groups = []
            c0 = 0
            for gi, gw in enumerate(WI_GROUPS):
                wi_groups.append(
                    wpool.tile([P, KH, gw * P], BF16, tag=f"wig{gi}", name=f"wig{gi}")
                )
                eng = nc.sync if gi % 2 == 0 else nc.scalar
                eng.dma_start(out=wi_groups[gi], in_=wiT_r[:, :, c0 : c0 + gw * P])
                c0 += gw * P

            wrT_sb = wpool.tile([P, KH, E], BF16)
            nc.sync.dma_start(out=wrT_sb, in_=wrT_r)
            bi_sb = wpool.tile([P, KF], F32)
            nc.scalar.dma_start(out=bi_sb, in_=bi_r)

            woT_sb = wpool.tile([P, KF, H], BF16)
            for g in range(0, KF, 4):
                eng = nc.scalar if (g // 4) % 2 == 0 else nc.sync
                eng.dma_start(out=woT_sb[:, g : g + 4, :], in_=woT_r[:, g : g + 4, :])
            # bo broadcast to all 128 partitions (it is added along the free dim)
            bo_sb = wpool.tile([P, H], F32)
            nc.gpsimd.dma_start(out=bo_sb, in_=bo[None, :].to_broadcast([P, H]))
            # one-hot expert selector, broadcast to all partitions
            esel_sb = wpool.tile([P, E], F32)
            nc.gpsimd.dma_start(out=esel_sb, in_=esel[None, :].to_broadcast([P, E]))
            # identity for the PE-mode transpose of the router logits
            id8 = wpool.tile([E, E], F32, name="id8")
            make_identity(nc, id8)

            def router_logits(x_bf, b):
                # logitsT [E, b] via the same bf16 x the FFN uses; fp32 psum.
                pslT = pr_pool.tile([E, b], F32, tag="pr")
                for k in range(KH):
                    nc.tensor.matmul(
                        pslT,
                        lhsT=wrT_sb[:, k, :],
                        rhs=x_bf[:, k, :],
                        start=(k == 0),
                        stop=(k == KH - 1),
                    )
                lgT_sb = rpool.tile([E, b], F32, tag="lgT")
                nc.vector.tensor_copy(out=lgT_sb, in_=pslT)
                return lgT_sb

            def router_chain(ts, ts0, tsz, lgT_sb, pst_blk, w_blk):
                # transpose this ts's logits back to [t, e] into a shared psum
                # tile. Only ts==0 uses start=True: it marks the whole 2KB
                # psum zero-region pending-zero, later transposes zero their
                # own bytes on first touch without wiping earlier columns.
                nc.tensor.matmul(
                    pst_blk[0:tsz, E * ts : E * (ts + 1)],
                    lhsT=lgT_sb[:, ts0 : ts0 + tsz],
                    rhs=id8,
                    is_transpose=True,
                    start=(ts == 0),
                    stop=True,
                    skip_group_check=True,
                )
                lg = pst_blk[0:tsz, E * ts : E * (ts + 1)]
                # top-2: m1 = max, m2 = max with the argmax masked out
                m1 = rpool.tile([P, 1], F32, tag="m1")
                nc.vector.reduce_max(m1[0:tsz], lg, axis=mybir.AxisListType.X)
                ge = rpool.tile([P, E], F32, tag="ge")
                nc.vector.tensor_scalar(
                    ge[0:tsz], lg, scalar1=m1[0:tsz], scalar2=-1e30,
                    op0=mybir.AluOpType.is_ge, op1=mybir.AluOpType.mult,
                )
                mk = rpool.tile([P, E], F32, tag="mk")
                nc.vector.tensor_tensor(mk[0:tsz], lg, ge[0:tsz], op=mybir.AluOpType.add)
                m2 = rpool.tile([P, 1], F32, tag="m2")
                nc.vector.reduce_max(m2[0:tsz], mk[0:tsz], axis=mybir.AxisListType.X)
                # this core's logit: lc = sum(lg * esel)
                lce = rpool.tile([P, E], F32, tag="lce")
                nc.vector.tensor_tensor(
                    lce[0:tsz], lg, esel_sb[0:tsz], op=mybir.AluOpType.mult
                )
                lc = rpool.tile([P, 1], F32, tag="lc")
                nc.vector.reduce_sum(lc[0:tsz], lce[0:tsz], axis=mybir.AxisListType.X)
                # w = 1 / (exp(m1-lc) + exp(m2-lc)); lc is m1 or m2 up to
                # rounding, so both args are in [-eps, m1-m2]: no overflow.
                aa = rpool.tile([P, 2], F32, tag="aa")
                nc.vector.tensor_tensor(
                    aa[0:tsz, 0:1], m1[0:tsz], lc[0:tsz], op=mybir.AluOpType.subtract
                )
                nc.vector.tensor_tensor(
                    aa[0:tsz, 1:2], m2[0:tsz], lc[0:tsz], op=mybir.AluOpType.subtract
                )
                ee = rpool.tile([P, 2], F32, tag="ee")
                nc.scalar.activation(ee[0:tsz], aa[0:tsz], mybir.ActivationFunctionType.Exp)
                den = rpool.tile([P, 1], F32, tag="den")
                nc.vector.tensor_tensor(
                    den[0:tsz], ee[0:tsz, 0:1], ee[0:tsz, 1:2], op=mybir.AluOpType.add
                )
                nc.vector.reciprocal(w_blk[0:tsz, ts : ts + 1], den[0:tsz])

            t0 = 0
            for ib, b in enumerate(blocks):
                # ts tiles within the block (last may be partial)
                ts_sizes = [P] * (b // P) + ([b % P] if b % P else [])
                ntiles = len(ts_sizes)

                x_bf = x_tiles.pop(ib)
                # prefetch next block's x (xpool bufs=3 -> no WAR stall)
                if ib + 1 < len(blocks):
                    bn = blocks[ib + 1]
                    tn = t0 + b
                    x_next = xpool.tile([P, KH, bn], BF16, tag="xb", name="x_next")
                    x_tiles[ib + 1] = x_next
                    nc.sync.dma_start(out=x_next, in_=xg_r[:, :, tn : tn + bn])

                w_blk = rpool.tile([P, ntiles], F32, tag="w")
                pst_blk = prt_pool.tile([P, E * ntiles], F32, tag="prt")

                # ---- layer 1: hT[f, t] = gelu(WiT^T @ xT + bi), with the
                # router work interleaved between the dense j-chains so the
                # PE activity stays dense ----
                hT = hpool.tile([P, KF, b], BF16, tag="hT")
                for j in range(KF):
                    gi, jj = j_map[j]
                    ps = ph_pool.tile([P, b], F32, tag="ph")
                    wig = wi_groups[gi]
                    for k in range(KH):
                        nc.tensor.matmul(
                            ps,
                            lhsT=wig[:, k, jj * P : (jj + 1) * P],
                            rhs=x_bf[:, k, :],
                            start=(k == 0),
                            stop=(k == KH - 1),
                        )
                    nc.scalar.activation(
                        out=hT[:, j, :],
                        in_=ps,
                        func=mybir.ActivationFunctionType.Gelu,
                        bias=bi_sb[:, j : j + 1],
                        scale=1.0,
                    )
                    if j == 1:
                        lgT_sb = router_logits(x_bf, b)
                    elif 2 <= j < 2 + ntiles:
                        ts = j - 2
                        router_chain(
                            ts, ts * P, ts_sizes[ts], lgT_sb, pst_blk, w_blk
                        )

                # ---- layer 2 + bo + routing-weight scale ----
                o_blk = opool.tile([P, ntiles, H], F32, tag="os")
                for ts in range(ntiles):
                    tsz = ts_sizes[ts]
                    po_a = po_pool.tile([P, HO], F32, tag="po")
                    po_b = po_pool.tile([P, HO], F32, tag="po")
                    for j in range(KF):
                        lhsT = hT[:, j, ts * P : ts * P + tsz]
                        nc.tensor.matmul(
                            po_a[0:tsz], lhsT=lhsT, rhs=woT_sb[:, j, 0:HO],
                            start=(j == 0), stop=(j == KF - 1),
                        )
                        nc.tensor.matmul(
                            po_b[0:tsz], lhsT=lhsT, rhs=woT_sb[:, j, HO : 2 * HO],
                            start=(j == 0), stop=(j == KF - 1),
                        )
                    o_sl = o_blk[0:tsz, ts, :]
                    wcol = w_blk[0:tsz, ts : ts + 1]
                    nc.vector.tensor_tensor(
                        o_sl[:, 0:HO], po_a[0:tsz], bo_sb[0:tsz, 0:HO],
                        op=mybir.AluOpType.add,
                    )
                    nc.vector.tensor_tensor(
                        o_sl[:, HO : 2 * HO], po_b[0:tsz], bo_sb[0:tsz, HO : 2 * HO],
                        op=mybir.AluOpType.add,
                    )
                    nc.vector.tensor_scalar_mul(o_sl, o_sl, scalar1=wcol)

                # one batched out DMA per block (tail tile separately: its
                # partition count differs)
                n0 = t0 // P
                nfull = b // P
                if nfull:
                    nc.sync.dma_start(
                        out=out_r[:, n0 : n0 + nfull, :],
                        in_=o_blk[:, 0:nfull, :],
                    )
                if b % P:
                    tsz = b % P
                    nc.sync.dma_start(
                        out=out_r[0:tsz, n0 + nfull, :],
                        in_=o_blk[0:tsz, nfull, :],
                    )
                t0 += b

    nc.compile()
    return nc


_NC_CACHE: dict = {}


def _get_nc(cap: int):
    if cap not in _NC_CACHE:
        _NC_CACHE[cap] = build_nc(cap)
    return _NC_CACHE[cap]


def _ensure_axon_hooks_module():
    """run_bass_kernel_spmd(trace=True) (e.g. via env BASS_TRACE=1) imports
    antenv.axon_hooks, which some images lack even though the boot code that
    would register the NTFF hook is present. Provide the module and register
    the real hook when available so tracing works instead of crashing."""
    try:
        import antenv.axon_hooks  # noqa: F401

        return
    except ImportError:
        pass
    try:
        import sys
        import types

        import antenv  # noqa: F401

        mod = types.ModuleType("antenv.axon_hooks")
        state = {"hook": None}
        mod.set_axon_ntff_profile_hook = lambda h: state.__setitem__("hook", h)
        mod.get_axon_ntff_profile_hook = lambda: state["hook"]
        try:
            from trn_agent_boot.trn_boot import _ntff_profile_via_ctypes

            mod.set_axon_ntff_profile_hook(
                _ntff_profile_via_ctypes("/opt/axon/libaxon_pjrt.so")
            )
        except Exception:
            pass
        sys.modules["antenv.axon_hooks"] = mod
    except Exception:
        pass


def _shard_tokens(xf, Wr):
    """Host-side sharding function: top-2 expert index per token (matches
    jax.lax.top_k tie-breaking: lowest index wins on ties)."""
    logits = xf.astype(np.float32) @ np.asarray(Wr, np.float32).T  # [T, E]
    i1 = np.argmax(logits, axis=1)
    l2 = logits.copy()
    l2[np.arange(len(i1)), i1] = -np.inf
    i2 = np.argmax(l2, axis=1)
    tokens = np.arange(logits.shape[0])
    tok_lists = []
    for c in range(N_CORES):
        tok_lists.append(np.concatenate([tokens[i1 == c], tokens[i2 == c]]))
    return tok_lists


def kernel(x, Wr, Wi, bi, Wo, bo, _trace=False):
    x = np.asarray(x)
    xf = x.reshape(-1, H).astype(np.float32)
    tok_lists = _shard_tokens(xf, Wr)
    cap = max(512, max(len(tl) for tl in tok_lists))

    xT = np.ascontiguousarray(xf.T).astype(BF16_NP)  # [H, T] bf16
    wrT_bf = np.ascontiguousarray(np.asarray(Wr, np.float32).T).astype(BF16_NP)
    in_maps = []
    for c in range(N_CORES):
        tl = tok_lists[c]
        xg = np.zeros((H, cap), dtype=BF16_NP)
        xg[:, : len(tl)] = xT[:, tl]
        sel = np.zeros(E, np.float32)
        sel[c] = 1.0
        in_maps.append(
            {
                "xg": xg,
                "wiT": np.ascontiguousarray(np.asarray(Wi[c], np.float32).T).astype(
                    BF16_NP
                ),
                "woT": np.ascontiguousarray(np.asarray(Wo[c], np.float32).T).astype(
                    BF16_NP
                ),
                "wrT": wrT_bf,
                "bi": np.asarray(bi[c], np.float32),
                "bo": np.asarray(bo[c], np.float32),
                "esel": sel,
            }
        )

    _ensure_axon_hooks_module()
    nc = _get_nc(cap)
    res = run_bass_kernel_spmd(
        nc, in_maps, core_ids=list(range(N_CORES)), trace=_trace
    )

    # Unshard: scatter-add the per-expert (already routing-weighted) rows.
    out = np.zeros((T, H), dtype=np.float32)
    for c in range(N_CORES):
        tl = tok_lists[c]
        out[tl] += res.results[c]["out"][: len(tl)]
    out = out.reshape(x.shape)
    if _trace:
        return out, res
    return out
